# revision 18
# baseline (speedup 1.0000x reference)
"""Multi-head attention (B=4, N=2048, E=512, H=8) on 8 Trainium2 cores.

Sharding: core c -> (batch b = c//2, head-group g = c%2 of 4 heads).
Each core computes q/k/v projections for its 4 heads, full attention,
and a partial output projection; the host sums the two partials per batch.

Column-split streaming design (all matmul operands fp16, PSUM f32):
  - projections: qT/kT per head-pair [128(2x64 dims), N]; v_aug[kc]
    [128(nk), 4*65] with a ones column per head (softmax denominator).
  - queries processed in two 1024-wide column halves. Per (half, pair,
    kc): energy chunk [128(nk), 1024] via K=64 row-tiled matmuls (the
    pair's heads run concurrently on partition halves), exp'd straight
    out of PSUM, then immediately consumed by att@v accumulation - attT
    tiles live only a few kc, so no big SBUF working set.
  - exp split between ACT (true Exp) and DVE (Schraudolph: i16 =
    round(1024*(log2e*SCALE*e + 15 - C)) bitcast to fp16 ~= exp(e*SCALE)
    within +-3% sawtooth; constant factor cancels in the softmax
    normalization, sawtooth averages out over the 2048-key contraction).
  - att@v: v_aug stationary [128, 65], attT moving (N=512); out [65, nq]
    accumulated over 16 kc in PSUM; row 64 = softmax denominator.
  - normalize on drain: DVE reciprocal_approx on the denom row, gpsimd
    partition_broadcast, DVE tensor_tensor multiply into oT fp16; odd
    heads land via a [64 x N/2] SBUF->SBUF DMA partition shift.
  - out projection: lhsT = oT pair chunks, rhs = Wo slices; fp16 out.
"""

import sys

if "/opt/trn_rl_repo" not in sys.path:
    sys.path.insert(0, "/opt/trn_rl_repo")

import numpy as np

B, N, E, H, D = 4, 2048, 512, 8, 64
NH = 4                      # heads per core
NKC = N // 128              # 16 key chunks
ECH = E // 128              # 4 embedding chunks
SCALE = float(1.0 / np.sqrt(E))
N_CORES = 8

# Schraudolph exp->fp16-bits constants: bits = 1024*(log2(e)*SCALE*x + 15 - C)
LOG2E = 1.4426950408889634
SCH_A = float(1024.0 * LOG2E * SCALE)
SCH_B = float(1024.0 * (15.0 - 0.0430))
ACT_FRAC = 0.55             # fraction of exp chunks handled by ScalarE

_built = None


def _build():
    global _built
    if _built is not None:
        return _built

    from contextlib import ExitStack

    import concourse.bass as bass  # noqa: F401
    import concourse.mybir as mybir
    import concourse.tile as tile
    from concourse import bacc

    DT = mybir.dt.float16
    F32 = mybir.dt.float32
    I16 = mybir.dt.int16
    AF = mybir.ActivationFunctionType
    ALU = mybir.AluOpType

    nc = bacc.Bacc(
        "TRN2",
        target_bir_lowering=False,
        debug=False,
        num_devices=N_CORES,
    )

    xqT = nc.dram_tensor("xqT", [E, N], DT, kind="ExternalInput").ap()
    xkT = nc.dram_tensor("xkT", [E, N], DT, kind="ExternalInput").ap()
    xvT = nc.dram_tensor("xvT", [E, N], DT, kind="ExternalInput").ap()
    wqt = nc.dram_tensor("wqt", [E, 256], DT, kind="ExternalInput").ap()
    wkt = nc.dram_tensor("wkt", [E, 256], DT, kind="ExternalInput").ap()
    wvt = nc.dram_tensor("wvt", [E, 256], DT, kind="ExternalInput").ap()
    wot = nc.dram_tensor("wot", [256, E], DT, kind="ExternalInput").ap()
    out = nc.dram_tensor("out", [N, E], DT, kind="ExternalOutput").ap()

    with tile.TileContext(nc) as tc, ExitStack() as ctx:
        consts = ctx.enter_context(tc.tile_pool(name="consts", bufs=1))
        xp = ctx.enter_context(tc.tile_pool(name="xp", bufs=12))
        qkp = ctx.enter_context(tc.tile_pool(name="qkp", bufs=1))
        atp = ctx.enter_context(tc.tile_pool(name="atp", bufs=16))
        vp = ctx.enter_context(tc.tile_pool(name="vp", bufs=1))
        otp = ctx.enter_context(tc.tile_pool(name="otp", bufs=1))
        stg = ctx.enter_context(tc.tile_pool(name="stg", bufs=2))
        ostage = ctx.enter_context(tc.tile_pool(name="ostage", bufs=3))

        # PSUM: ps_e 2 x [128,1024]f32 (2 banks each) + ps_av 4 x 1 bank = 8
        ps_e = ctx.enter_context(tc.tile_pool(name="ps_e", bufs=2, space="PSUM"))
        ps_av = ctx.enter_context(tc.tile_pool(name="ps_av", bufs=4, space="PSUM"))

        # ---- weight loads ----
        wq_sb = [consts.tile([128, 256], DT, tag=f"wq{e}", name=f"wq_sb{e}") for e in range(ECH)]
        wk_sb = [consts.tile([128, 256], DT, tag=f"wk{e}", name=f"wk_sb{e}") for e in range(ECH)]
        wv_sb = [consts.tile([128, 256], DT, tag=f"wv{e}", name=f"wv_sb{e}") for e in range(ECH)]
        wo_sb = [consts.tile([128, E], DT, tag=f"wo{c}", name=f"wo_sb{c}") for c in range(2)]
        for e in range(ECH):
            nc.sync.dma_start(out=wq_sb[e][:], in_=wqt[128 * e:128 * (e + 1), :])
            nc.sync.dma_start(out=wk_sb[e][:], in_=wkt[128 * e:128 * (e + 1), :])
            nc.sync.dma_start(out=wv_sb[e][:], in_=wvt[128 * e:128 * (e + 1), :])
        for c in range(2):
            nc.sync.dma_start(out=wo_sb[c][:], in_=wot[128 * c:128 * (c + 1), :])

        # ---- activation inputs (transposed on host); chain q -> k -> v ----
        xq_sb, xk_sb, xv_sb = [], [], []
        for (src_ap, outl) in ((xqT, xq_sb), (xkT, xk_sb), (xvT, xv_sb)):
            for e in range(ECH):
                t = xp.tile([128, N], DT, tag="x", name="xin")
                nc.sync.dma_start(out=t[:], in_=src_ap[128 * e:128 * (e + 1), :])
                outl.append(t)

        # qT/kT per head pair p: head 2p on partitions 0:64, 2p+1 on 64:128
        qT = [qkp.tile([128, N], DT, tag=f"qT{p}", name=f"qT{p}") for p in range(2)]
        kT = [qkp.tile([128, N], DT, tag=f"kT{p}", name=f"kT{p}") for p in range(2)]
        vsb = [vp.tile([128, NH * 65], DT, tag=f"v{kc}", name=f"v_sb{kc}") for kc in range(NKC)]
        oT = [otp.tile([128, N], DT, tag=f"oT{p}", name=f"oT{p}") for p in range(2)]
        oTb = [otp.tile([64, N], DT, tag=f"oTb{p}", name=f"oTb{p}") for p in range(2)]

        copy_flip = [0]

        def eng_copy(dst, src):
            """Alternate proj-phase PSUM->SBUF copies between ACT and DVE."""
            if copy_flip[0] % 2 == 0:
                nc.scalar.copy(dst, src)
            else:
                nc.vector.tensor_copy(dst, src)
            copy_flip[0] += 1

        def emit_warm(n_mm=14):
            ps = ps_e.tile([128, 256], F32, tag="e", name="warm")
            for _ in range(n_mm):
                nc.tensor.matmul(ps[:], wq_sb[0][:, 0:128], wq_sb[0][:], start=True, stop=True)

        def emit_proj_qk(p):
            for (w_sb, x_sb, dst) in ((wq_sb, xq_sb, qT[p]), (wk_sb, xk_sb, kT[p])):
                for nsp in range(2):
                    ps = ps_e.tile([128, 1024], F32, tag="e", name="psqk")
                    for e in range(ECH):
                        for j in range(2):
                            nc.tensor.matmul(
                                ps[:, 512 * j:512 * (j + 1)],
                                w_sb[e][:, 128 * p:128 * (p + 1)],
                                x_sb[e][:, 1024 * nsp + 512 * j:1024 * nsp + 512 * (j + 1)],
                                start=(e == 0),
                                stop=(e == ECH - 1),
                            )
                    eng_copy(dst[:, 1024 * nsp:1024 * (nsp + 1)], ps[:])

        def emit_vproj():
            for kc in range(NKC):
                ps = ps_e.tile([128, 256], F32, tag="e", name="psv")
                for e in range(ECH):
                    nc.tensor.matmul(
                        ps[:],
                        xv_sb[e][:, 128 * kc:128 * (kc + 1)],
                        wv_sb[e][:],
                        start=(e == 0),
                        stop=(e == ECH - 1),
                    )
                t = vsb[kc]
                vdst = t[:].rearrange("p (h d) -> p h d", h=NH)[:, :, 0:D]
                vsrc = ps[:].rearrange("p (h d) -> p h d", h=NH)
                eng_copy(vdst, vsrc)
                ones_cols = t[:].rearrange("p (h d) -> p h d", h=NH)[:, :, D:D + 1]
                nc.vector.memset(ones_cols, 1.0)

        # ---- exp engine assignment ----
        exp_counter = [0, 0.0]  # [chunks emitted, chunks given to ACT]

        def emit_exp(ps, dst_ap):
            i, acta = exp_counter
            if acta < ACT_FRAC * (i + 1):
                nc.scalar.activation(dst_ap, ps[:], AF.Exp, scale=SCALE)
                exp_counter[1] += 1.0
            else:
                nc.vector.tensor_scalar(
                    dst_ap.bitcast(I16), ps[:], SCH_A, SCH_B, ALU.mult, ALU.add
                )
            exp_counter[0] += 1

        def emit_drains(accs, p, half):
            """Normalize the 4 accumulators of a (half, pair) block into oT
            fp16. Accumulators evacuate to SBUF staging immediately (split
            ACT/DVE) so their PSUM banks free for the next block. 1/denom via
            XOR-magic seed + 2 Newton iterations (standard DVE ops, batched
            at 32-aligned partitions); sign flipped, compensated by negating
            Wo on the host. Odd head rows reach partitions 64:128 of oT via
            a SBUF->SBUF DMA shift."""
            I32 = mybir.dt.int32
            dens = stg.tile([128, 512], F32, tag="dens", name="dens")
            stage = [[None, None], [None, None]]
            for r in range(2):
                for nsj in range(2):
                    idx = 2 * r + nsj
                    st = stg.tile([64, 512], F32, tag=f"sg{idx}", name="stage")
                    stage[r][nsj] = st
                    if r == 0:
                        nc.scalar.copy(st[:], accs[r][nsj][0:64, :])
                        nc.scalar.copy(dens[32 * idx:32 * idx + 1, :], accs[r][nsj][64:65, :])
                    else:
                        nc.vector.tensor_copy(st[:], accs[r][nsj][0:64, :])
                        nc.vector.tensor_copy(dens[32 * idx:32 * idx + 1, :], accs[r][nsj][64:65, :])
            y0 = stg.tile([128, 512], F32, tag="y0", name="y0")
            t1 = stg.tile([128, 512], F32, tag="t1", name="t1")
            nc.vector.tensor_scalar(
                y0[:].bitcast(I32), dens[:].bitcast(I32), -1, int(0x7EF312AC),
                ALU.mult, ALU.add,
            )
            nc.vector.tensor_mul(t1[:], dens[:], y0[:])
            nc.vector.scalar_tensor_tensor(t1[:], t1[:], 2.0, y0[:], op0=ALU.subtract, op1=ALU.mult)
            nc.vector.tensor_scalar(y0[:], t1[:], -1.0, None, ALU.mult)
            nc.vector.tensor_mul(t1[:], dens[:], y0[:])
            nc.vector.scalar_tensor_tensor(t1[:], t1[:], 2.0, y0[:], op0=ALU.subtract, op1=ALU.mult)
            # t1 rows 32*idx hold -1/denom(idx)
            for r in range(2):
                for nsj in range(2):
                    idx = 2 * r + nsj
                    rcp1 = stg.tile([1, 512], F32, tag="rcp1", name="rcp1")
                    nc.vector.tensor_copy(rcp1[:], t1[32 * idx:32 * idx + 1, :])
                    rb = stg.tile([64, 512], F32, tag="rb", name="rb")
                    nc.gpsimd.partition_broadcast(rb[:], rcp1[:])
                    dst = oT[p] if r == 0 else oTb[p]
                    col0 = 1024 * half + 512 * nsj
                    nc.vector.tensor_tensor(
                        dst[0:64, col0:col0 + 512], stage[r][nsj][:], rb[:], op=ALU.mult
                    )

        def emit_shift_b(p, half):
            nc.gpsimd.dma_start(
                out=oT[p][64:128, 1024 * half:1024 * (half + 1)],
                in_=oTb[p][0:64, 1024 * half:1024 * (half + 1)],
            )

        def emit_half_pair(half, p):
            """E + exp + att@v for head pair p over query columns
            [1024*half, 1024*(half+1)), streamed per key chunk."""
            LEAD = 6  # attv trails E/exp by this many key chunks
            att = [[None] * NKC, [None] * NKC]   # [r][kc]
            accs = [[None, None], [None, None]]  # [r][nsj]
            for r in range(2):
                for nsj in range(2):
                    accs[r][nsj] = ps_av.tile([65, 512], F32, tag="av", name="pav")

            def emit_e(kc):
                for r in range(2):
                    ps = ps_e.tile([128, 1024], F32, tag="e", name="pse")
                    lo = 64 * r
                    for j in range(2):
                        nc.tensor.matmul(
                            ps[:, 512 * j:512 * (j + 1)],
                            kT[p][lo:lo + 64, 128 * kc:128 * (kc + 1)],
                            qT[p][lo:lo + 64, 1024 * half + 512 * j:1024 * half + 512 * (j + 1)],
                            start=True,
                            stop=True,
                        )
                    t = atp.tile([128, 1024], DT, tag="at", name="attT")
                    att[r][kc] = t
                    emit_exp(ps, t[:])

            def emit_av(kc):
                for r in range(2):
                    h = 2 * p + r
                    t = att[r][kc]
                    for nsj in range(2):
                        nc.tensor.matmul(
                            accs[r][nsj][:],
                            vsb[kc][:, 65 * h:65 * h + 65],
                            t[:, 512 * nsj:512 * (nsj + 1)],
                            start=(kc == 0),
                            stop=(kc == NKC - 1),
                        )

            for kc in range(NKC):
                emit_e(kc)
                if kc >= LEAD:
                    emit_av(kc - LEAD)
            for kc in range(NKC - LEAD, NKC):
                emit_av(kc)
            emit_drains(accs, p, half)

        def emit_oproj(m_list):
            for m in m_list:
                po = ps_av.tile([128, 512], F32, tag="av", name="po")
                for c in range(2):
                    nc.tensor.matmul(
                        po[:],
                        oT[c][:, 128 * m:128 * (m + 1)],
                        wo_sb[c][:],
                        start=(c == 0),
                        stop=(c == 1),
                    )
                st = ostage.tile([128, E], DT, tag="st", name="st")
                eng_copy(st[:], po[:])
                nc.sync.dma_start(out=out[128 * m:128 * (m + 1), :], in_=st[:])

        # ---- schedule ----
        emit_warm()
        emit_proj_qk(0)
        emit_proj_qk(1)
        emit_vproj()
        for half in range(2):
            for p in range(2):
                emit_half_pair(half, p)
                emit_shift_b(p, half)
            if half == 0:
                emit_oproj(range(0, 8))
        emit_oproj(range(8, NKC))

    nc.compile()
    _built = nc
    return nc


def _host_prep(query, key, value, Wq, Wk, Wv, Wo, c):
    b, g = c // 2, c % 2
    DT = np.float16
    rows = slice(256 * g, 256 * (g + 1))
    return {
        "xqT": np.ascontiguousarray(query[b].T).astype(DT),
        "xkT": np.ascontiguousarray(key[b].T).astype(DT),
        "xvT": np.ascontiguousarray(value[b].T).astype(DT),
        "wqt": np.ascontiguousarray(Wq[rows, :].T).astype(DT),
        "wkt": np.ascontiguousarray(Wk[rows, :].T).astype(DT),
        "wvt": np.ascontiguousarray(Wv[rows, :].T).astype(DT),
        "wot": np.ascontiguousarray(-Wo[:, rows].T).astype(DT),
    }


# test.py can flip these to profile
TRACE = False
TRACE_KWARGS = {}
LAST_RESULTS = None


def kernel(query, key, value, Wq, Wk, Wv, Wo):
    global LAST_RESULTS
    from concourse.bass_utils import run_bass_kernel_spmd

    args = [np.asarray(x, dtype=np.float32) for x in (query, key, value, Wq, Wk, Wv, Wo)]
    nc = _build()
    in_maps = [_host_prep(*args, c) for c in range(N_CORES)]
    res = run_bass_kernel_spmd(
        nc, in_maps, core_ids=list(range(N_CORES)), trace=TRACE, **TRACE_KWARGS
    )
    LAST_RESULTS = res
    outp = np.zeros((B, N, E), np.float32)
    for c in range(N_CORES):
        outp[c // 2] += res.results[c]["out"].astype(np.float32)
    return outp


# revision 20
# speedup vs baseline: 1.0737x; 1.0737x over previous
"""Multi-head attention (B=4, N=2048, E=512, H=8) on 8 Trainium2 cores.

Sharding: core c -> (batch b = c//2, head-group g = c%2 of 4 heads).
Each core computes q/k/v projections for its 4 heads, full attention,
and a partial output projection; the host sums the two partials per batch.

Column-split streaming design (all matmul operands fp16, PSUM f32):
  - projections: qT/kT per head-pair [128(2x64 dims), N]; v_aug[kc]
    [128(nk), 4*65] with a ones column per head (softmax denominator).
  - queries processed in two 1024-wide column halves. Per (half, pair,
    kc): energy chunk [128(nk), 1024] via K=64 row-tiled matmuls (the
    pair's heads run concurrently on partition halves), exp'd straight
    out of PSUM, then immediately consumed by att@v accumulation - attT
    tiles live only a few kc, so no big SBUF working set.
  - exp split between ACT (true Exp) and DVE (Schraudolph: i16 =
    round(1024*(log2e*SCALE*e + 15 - C)) bitcast to fp16 ~= exp(e*SCALE)
    within +-3% sawtooth; constant factor cancels in the softmax
    normalization, sawtooth averages out over the 2048-key contraction).
  - att@v: v_aug stationary [128, 65], attT moving (N=512); out [65, nq]
    accumulated over 16 kc in PSUM; row 64 = softmax denominator.
  - normalize on drain: DVE reciprocal_approx on the denom row, gpsimd
    partition_broadcast, DVE tensor_tensor multiply into oT fp16; odd
    heads land via a [64 x N/2] SBUF->SBUF DMA partition shift.
  - out projection: lhsT = oT pair chunks, rhs = Wo slices; fp16 out.
"""

import sys

if "/opt/trn_rl_repo" not in sys.path:
    sys.path.insert(0, "/opt/trn_rl_repo")

import numpy as np

B, N, E, H, D = 4, 2048, 512, 8, 64
NH = 4                      # heads per core
NKC = N // 128              # 16 key chunks
ECH = E // 128              # 4 embedding chunks
SCALE = float(1.0 / np.sqrt(E))
N_CORES = 8

# Schraudolph exp->fp16-bits constants: bits = 1024*(log2(e)*SCALE*x + 15 - C)
LOG2E = 1.4426950408889634
SCH_A = float(1024.0 * LOG2E * SCALE)
SCH_B = float(1024.0 * (15.0 - 0.0430))
ACT_FRAC = 0.55             # fraction of exp chunks handled by ScalarE

_built = None


def _build():
    global _built
    if _built is not None:
        return _built

    from contextlib import ExitStack

    import concourse.bass as bass  # noqa: F401
    import concourse.mybir as mybir
    import concourse.tile as tile
    from concourse import bacc

    DT = mybir.dt.float16
    F32 = mybir.dt.float32
    I16 = mybir.dt.int16
    AF = mybir.ActivationFunctionType
    ALU = mybir.AluOpType

    nc = bacc.Bacc(
        "TRN2",
        target_bir_lowering=False,
        debug=False,
        num_devices=N_CORES,
    )

    xqT = nc.dram_tensor("xqT", [E, N], DT, kind="ExternalInput").ap()
    xkT = nc.dram_tensor("xkT", [E, N], DT, kind="ExternalInput").ap()
    xvT = nc.dram_tensor("xvT", [E, N], DT, kind="ExternalInput").ap()
    wqt = nc.dram_tensor("wqt", [E, 256], DT, kind="ExternalInput").ap()
    wkt = nc.dram_tensor("wkt", [E, 256], DT, kind="ExternalInput").ap()
    wvt = nc.dram_tensor("wvt", [E, 256], DT, kind="ExternalInput").ap()
    wot = nc.dram_tensor("wot", [256, E], DT, kind="ExternalInput").ap()
    out = nc.dram_tensor("out", [N, E], DT, kind="ExternalOutput").ap()

    with tile.TileContext(nc) as tc, ExitStack() as ctx:
        consts = ctx.enter_context(tc.tile_pool(name="consts", bufs=1))
        xp = ctx.enter_context(tc.tile_pool(name="xp", bufs=12))
        qkp = ctx.enter_context(tc.tile_pool(name="qkp", bufs=1))
        atp = ctx.enter_context(tc.tile_pool(name="atp", bufs=16))
        vp = ctx.enter_context(tc.tile_pool(name="vp", bufs=1))
        otp = ctx.enter_context(tc.tile_pool(name="otp", bufs=1))
        stg = ctx.enter_context(tc.tile_pool(name="stg", bufs=2))
        ostage = ctx.enter_context(tc.tile_pool(name="ostage", bufs=3))

        # PSUM: ps_e 2 x [128,1024]f32 (2 banks each) + ps_av 4 x 1 bank = 8
        ps_e = ctx.enter_context(tc.tile_pool(name="ps_e", bufs=4, space="PSUM"))
        ps_av = ctx.enter_context(tc.tile_pool(name="ps_av", bufs=4, space="PSUM"))

        # ---- weight loads ----
        wq_sb = [consts.tile([128, 256], DT, tag=f"wq{e}", name=f"wq_sb{e}") for e in range(ECH)]
        wk_sb = [consts.tile([128, 256], DT, tag=f"wk{e}", name=f"wk_sb{e}") for e in range(ECH)]
        wv_sb = [consts.tile([128, 256], DT, tag=f"wv{e}", name=f"wv_sb{e}") for e in range(ECH)]
        wo_sb = [consts.tile([128, E], DT, tag=f"wo{c}", name=f"wo_sb{c}") for c in range(2)]
        for e in range(ECH):
            nc.sync.dma_start(out=wq_sb[e][:], in_=wqt[128 * e:128 * (e + 1), :])
            nc.sync.dma_start(out=wk_sb[e][:], in_=wkt[128 * e:128 * (e + 1), :])
            nc.sync.dma_start(out=wv_sb[e][:], in_=wvt[128 * e:128 * (e + 1), :])
        for c in range(2):
            nc.sync.dma_start(out=wo_sb[c][:], in_=wot[128 * c:128 * (c + 1), :])

        # ---- activation inputs (transposed on host); chain q -> k -> v ----
        xq_sb, xk_sb, xv_sb = [], [], []
        for (src_ap, outl) in ((xqT, xq_sb), (xkT, xk_sb), (xvT, xv_sb)):
            for e in range(ECH):
                t = xp.tile([128, N], DT, tag="x", name="xin")
                nc.sync.dma_start(out=t[:], in_=src_ap[128 * e:128 * (e + 1), :])
                outl.append(t)

        # qT/kT per head pair p: head 2p on partitions 0:64, 2p+1 on 64:128
        qT = [qkp.tile([128, N], DT, tag=f"qT{p}", name=f"qT{p}") for p in range(2)]
        kT = [qkp.tile([128, N], DT, tag=f"kT{p}", name=f"kT{p}") for p in range(2)]
        vsb = [vp.tile([128, NH * 65], DT, tag=f"v{kc}", name=f"v_sb{kc}") for kc in range(NKC)]
        oT = [otp.tile([128, N], DT, tag=f"oT{p}", name=f"oT{p}") for p in range(2)]
        oTb = [otp.tile([64, N], DT, tag=f"oTb{p}", name=f"oTb{p}") for p in range(2)]

        copy_flip = [0]

        def eng_copy(dst, src):
            """Alternate proj-phase PSUM->SBUF copies between ACT and DVE."""
            if copy_flip[0] % 2 == 0:
                nc.scalar.copy(dst, src)
            else:
                nc.vector.tensor_copy(dst, src)
            copy_flip[0] += 1

        def emit_warm(n_mm=14):
            ps = ps_e.tile([128, 256], F32, tag="e", name="warm")
            for _ in range(n_mm):
                nc.tensor.matmul(ps[:], wq_sb[0][:, 0:128], wq_sb[0][:], start=True, stop=True)

        def emit_proj_qk(p):
            for (w_sb, x_sb, dst) in ((wq_sb, xq_sb, qT[p]), (wk_sb, xk_sb, kT[p])):
                for ns in range(4):
                    ps = ps_e.tile([128, 512], F32, tag="e", name="psqk")
                    for e in range(ECH):
                        nc.tensor.matmul(
                            ps[:],
                            w_sb[e][:, 128 * p:128 * (p + 1)],
                            x_sb[e][:, 512 * ns:512 * (ns + 1)],
                            start=(e == 0),
                            stop=(e == ECH - 1),
                        )
                    eng_copy(dst[:, 512 * ns:512 * (ns + 1)], ps[:])

        def emit_vproj():
            for kc in range(NKC):
                ps = ps_e.tile([128, 256], F32, tag="e", name="psv")
                for e in range(ECH):
                    nc.tensor.matmul(
                        ps[:],
                        xv_sb[e][:, 128 * kc:128 * (kc + 1)],
                        wv_sb[e][:],
                        start=(e == 0),
                        stop=(e == ECH - 1),
                    )
                t = vsb[kc]
                vdst = t[:].rearrange("p (h d) -> p h d", h=NH)[:, :, 0:D]
                vsrc = ps[:].rearrange("p (h d) -> p h d", h=NH)
                eng_copy(vdst, vsrc)
                ones_cols = t[:].rearrange("p (h d) -> p h d", h=NH)[:, :, D:D + 1]
                nc.vector.memset(ones_cols, 1.0)

        # ---- exp engine assignment ----
        exp_counter = [0, 0.0]  # [chunks emitted, chunks given to ACT]

        def emit_exp(ps, dst_ap):
            i, acta = exp_counter
            if acta < ACT_FRAC * (i + 1):
                nc.scalar.activation(dst_ap, ps[:], AF.Exp, scale=SCALE)
                exp_counter[1] += 1.0
            else:
                nc.vector.tensor_scalar(
                    dst_ap.bitcast(I16), ps[:], SCH_A, SCH_B, ALU.mult, ALU.add
                )
            exp_counter[0] += 1

        def emit_drains(accs, p, half):
            """Normalize the 4 accumulators of a (half, pair) block into oT
            fp16. Accumulators evacuate to SBUF staging immediately (split
            ACT/DVE) so their PSUM banks free for the next block. 1/denom via
            XOR-magic seed + 2 Newton iterations (standard DVE ops, batched
            at 32-aligned partitions); sign flipped, compensated by negating
            Wo on the host. Odd head rows reach partitions 64:128 of oT via
            a SBUF->SBUF DMA shift."""
            I32 = mybir.dt.int32
            dens = stg.tile([128, 512], F32, tag="dens", name="dens")
            stage = [[None, None], [None, None]]
            for r in range(2):
                for nsj in range(2):
                    idx = 2 * r + nsj
                    st = stg.tile([64, 512], F32, tag=f"sg{idx}", name="stage")
                    stage[r][nsj] = st
                    if r == 0:
                        nc.scalar.copy(st[:], accs[r][nsj][0:64, :])
                        nc.scalar.copy(dens[32 * idx:32 * idx + 1, :], accs[r][nsj][64:65, :])
                    else:
                        nc.vector.tensor_copy(st[:], accs[r][nsj][0:64, :])
                        nc.vector.tensor_copy(dens[32 * idx:32 * idx + 1, :], accs[r][nsj][64:65, :])
            y0 = stg.tile([128, 512], F32, tag="y0", name="y0")
            t1 = stg.tile([128, 512], F32, tag="t1", name="t1")
            nc.vector.tensor_scalar(
                y0[:].bitcast(I32), dens[:].bitcast(I32), -1, int(0x7EF312AC),
                ALU.mult, ALU.add,
            )
            nc.vector.tensor_mul(t1[:], dens[:], y0[:])
            nc.vector.scalar_tensor_tensor(t1[:], t1[:], 2.0, y0[:], op0=ALU.subtract, op1=ALU.mult)
            nc.vector.tensor_scalar(y0[:], t1[:], -1.0, None, ALU.mult)
            nc.vector.tensor_mul(t1[:], dens[:], y0[:])
            nc.vector.scalar_tensor_tensor(t1[:], t1[:], 2.0, y0[:], op0=ALU.subtract, op1=ALU.mult)
            # t1 rows 32*idx hold -1/denom(idx)
            for r in range(2):
                for nsj in range(2):
                    idx = 2 * r + nsj
                    rcp1 = stg.tile([1, 512], F32, tag="rcp1", name="rcp1")
                    nc.vector.tensor_copy(rcp1[:], t1[32 * idx:32 * idx + 1, :])
                    rb = stg.tile([64, 512], F32, tag="rb", name="rb")
                    nc.gpsimd.partition_broadcast(rb[:], rcp1[:])
                    dst = oT[p] if r == 0 else oTb[p]
                    col0 = 1024 * half + 512 * nsj
                    nc.vector.tensor_tensor(
                        dst[0:64, col0:col0 + 512], stage[r][nsj][:], rb[:], op=ALU.mult
                    )

        def emit_shift_b(p, half):
            nc.gpsimd.dma_start(
                out=oT[p][64:128, 1024 * half:1024 * (half + 1)],
                in_=oTb[p][0:64, 1024 * half:1024 * (half + 1)],
            )

        def emit_half_pair(half, p):
            """E + exp + att@v for head pair p over query columns
            [1024*half, 1024*(half+1)), streamed per key chunk."""
            LEAD = 6  # attv trails E/exp by this many key chunks
            att = [[None] * NKC, [None] * NKC]   # [r][kc]
            accs = [[None, None], [None, None]]  # [r][nsj]
            for r in range(2):
                for nsj in range(2):
                    accs[r][nsj] = ps_av.tile([65, 512], F32, tag="av", name="pav")

            def emit_e(kc):
                for r in range(2):
                    lo = 64 * r
                    t = atp.tile([128, 1024], DT, tag="at", name="attT")
                    att[r][kc] = t
                    for j in range(2):
                        ps = ps_e.tile([128, 512], F32, tag="e", name="pse")
                        nc.tensor.matmul(
                            ps[:],
                            kT[p][lo:lo + 64, 128 * kc:128 * (kc + 1)],
                            qT[p][lo:lo + 64, 1024 * half + 512 * j:1024 * half + 512 * (j + 1)],
                            start=True,
                            stop=True,
                        )
                        emit_exp(ps, t[:, 512 * j:512 * (j + 1)])

            def emit_av(kc):
                for r in range(2):
                    h = 2 * p + r
                    t = att[r][kc]
                    for nsj in range(2):
                        nc.tensor.matmul(
                            accs[r][nsj][:],
                            vsb[kc][:, 65 * h:65 * h + 65],
                            t[:, 512 * nsj:512 * (nsj + 1)],
                            start=(kc == 0),
                            stop=(kc == NKC - 1),
                        )

            for kc in range(NKC):
                emit_e(kc)
                if kc >= LEAD:
                    emit_av(kc - LEAD)
            for kc in range(NKC - LEAD, NKC):
                emit_av(kc)
            emit_drains(accs, p, half)

        def emit_oproj(m_list):
            for m in m_list:
                po = ps_av.tile([128, 512], F32, tag="av", name="po")
                for c in range(2):
                    nc.tensor.matmul(
                        po[:],
                        oT[c][:, 128 * m:128 * (m + 1)],
                        wo_sb[c][:],
                        start=(c == 0),
                        stop=(c == 1),
                    )
                st = ostage.tile([128, E], DT, tag="st", name="st")
                eng_copy(st[:], po[:])
                nc.sync.dma_start(out=out[128 * m:128 * (m + 1), :], in_=st[:])

        # ---- schedule ----
        emit_warm()
        emit_proj_qk(0)
        emit_proj_qk(1)
        emit_vproj()
        for half in range(2):
            for p in range(2):
                emit_half_pair(half, p)
                emit_shift_b(p, half)
        emit_oproj(range(NKC))

    nc.compile()
    _built = nc
    return nc


def _host_prep(query, key, value, Wq, Wk, Wv, Wo, c):
    b, g = c // 2, c % 2
    DT = np.float16
    rows = slice(256 * g, 256 * (g + 1))
    return {
        "xqT": np.ascontiguousarray(query[b].T).astype(DT),
        "xkT": np.ascontiguousarray(key[b].T).astype(DT),
        "xvT": np.ascontiguousarray(value[b].T).astype(DT),
        "wqt": np.ascontiguousarray(Wq[rows, :].T).astype(DT),
        "wkt": np.ascontiguousarray(Wk[rows, :].T).astype(DT),
        "wvt": np.ascontiguousarray(Wv[rows, :].T).astype(DT),
        "wot": np.ascontiguousarray(-Wo[:, rows].T).astype(DT),
    }


# test.py can flip these to profile
TRACE = False
TRACE_KWARGS = {}
LAST_RESULTS = None


def kernel(query, key, value, Wq, Wk, Wv, Wo):
    global LAST_RESULTS
    from concourse.bass_utils import run_bass_kernel_spmd

    args = [np.asarray(x, dtype=np.float32) for x in (query, key, value, Wq, Wk, Wv, Wo)]
    nc = _build()
    in_maps = [_host_prep(*args, c) for c in range(N_CORES)]
    res = run_bass_kernel_spmd(
        nc, in_maps, core_ids=list(range(N_CORES)), trace=TRACE, **TRACE_KWARGS
    )
    LAST_RESULTS = res
    outp = np.zeros((B, N, E), np.float32)
    for c in range(N_CORES):
        outp[c // 2] += res.results[c]["out"].astype(np.float32)
    return outp
